# revision 18
# baseline (speedup 1.0000x reference)
"""Trainium2 Bass kernel for the histogram-binning bigram loss.

Math: reference returns (loss, gold) with
  gold = start[0] + end[-1] + sum_i B[i, i+1]
  loss = -gold + (1/S) * ( sum_s start[v_s0] + sum_s end[v_s,-1]
                           + sum_{s,j} B[v_sj, v_s,j+1] )

Strategy (8 cores, SPMD, w-slice sharding):
  Each core owns a 512-row window W_k = [512k, 512k+512) of B.
  For every sample (a permutation of [0,4096)) it builds the successor
  table nxt[s, w] = value following w in sample s, restricted to w in W_k,
  via one GPSIMD local_scatter per 128-sample round (indices are unique
  per partition because rows are permutations; successor of the last
  element is the sentinel 4096).
  Then sum_{s, w in W_k} B[w, nxt[s,w]] is computed as a histogram dot:
  split c = 64*hi + lo and accumulate C_w = OHhi_w^T @ OHlo_w on the
  tensor engine in PSUM; finally sum_w <C_w, B_w> on DVE.

  One-hots are built in bf16 with an (l-major, w-minor) free layout so
  every DVE operand has a packed 2-byte last dim -> 2x DVE perf mode
  (half the cycles of the fp8 build).  The tensor engine reads the SAME
  bytes through an fp8 view: bf16(1.0) = 0x3F80 = bytes (-0.0, 1.875) in
  fp8e4.  A DoubleRow matmul pairs byte-1 of two different rounds as its
  two k-tiles, so each hit contributes exactly 1.875^2 = 3.515625 and
  the count matrix comes out scaled by 3.515625 (divided out on host).
  start/end terms use a small fp8 one-hot histogram on the first/last
  columns of the samples, windowed by value so each core counts only its
  own rows.  Host just shards inputs, sums the 8 partial scalars and
  combines.
"""

import sys

import numpy as np

try:
    import concourse  # noqa: F401
except ImportError:  # pragma: no cover
    sys.path.insert(0, "/opt/trn_rl_repo")

N_WORDS = 4096
N_SAMPLES = 2048
N_CORES = 8
WSLICE = N_WORDS // N_CORES          # 512 rows of B per core
ROUNDS = N_SAMPLES // 128            # 16 sample rounds of 128
SENTINEL = N_WORDS                   # successor sentinel: hi=64, lo=0
WG = 32                              # w per PSUM supergroup
N_WG = WSLICE // WG                  # 16 supergroups
OH_SCALE = 1.875 * 1.875             # fp8 view of bf16(1.0) contributes this
GP_POOL_P = (1, 3, 5)                # (wg,p) slots whose one-hots are built
                                     # by GPSIMD local_scatter

_CACHE = {}


def _build_module():
    import concourse.bass as bass
    import concourse.bacc as bacc
    import concourse.tile as tile
    from concourse import mybir

    dt = mybir.dt
    Alu = mybir.AluOpType

    nc = bacc.Bacc()

    samples_d = nc.declare_dram_parameter(
        "samples", [N_SAMPLES, N_WORDS], dt.int16, isOutput=False)
    bslice_d = nc.declare_dram_parameter(
        "bslice", [WSLICE, N_WORDS], dt.float32, isOutput=False)
    wofs_d = nc.declare_dram_parameter(
        "wofs", [128, 1], dt.float32, isOutput=False)
    startv_d = nc.declare_dram_parameter(
        "startv", [1, WSLICE], dt.float32, isOutput=False)
    endv_d = nc.declare_dram_parameter(
        "endv", [1, WSLICE], dt.float32, isOutput=False)
    diag_d = nc.declare_dram_parameter(
        "diag", [1, N_WORDS], dt.float32, isOutput=False)
    partial_d = nc.declare_dram_parameter(
        "partial", [1, 4], dt.float32, isOutput=True)

    with tile.TileContext(nc) as tc:
        with (
            tc.tile_pool(name="persist", bufs=1) as persist,
            tc.tile_pool(name="work", bufs=2) as work,
            tc.tile_pool(name="oh", bufs=2) as ohp,
            tc.tile_pool(name="b3", bufs=2) as b3p,
            tc.tile_pool(name="drain", bufs=2) as drp,
            tc.tile_pool(name="psum", bufs=1, space="PSUM") as psum,
            tc.tile_pool(name="psc", bufs=1, space="PSUM") as psc,
        ):
            # ---- constants ----
            wofs_sb = persist.tile([128, 1], dt.float32)
            nc.gpsimd.dma_start(wofs_sb[:], wofs_d[:])
            iota64 = persist.tile([128, 64], dt.int16)
            nc.gpsimd.iota(iota64[:], pattern=[[1, 64]], base=0,
                           channel_multiplier=0)
            # iotaLHW[p, (l, hl, w)] = l
            iotaLHW = persist.tile([128, 64 * 2 * WG], dt.int16)
            nc.vector.tensor_copy(
                iotaLHW[:].rearrange("p (l hl w) -> p l hl w", l=64, hl=2),
                iota64[:].unsqueeze(2).unsqueeze(3).to_broadcast(
                    [128, 64, 2, WG]))
            ones128 = persist.tile([128, 1], dt.float32)
            nc.vector.memset(ones128[:], 1.0)
            # iotaW8HL[p, hl*32 + w] = (w % 8) * 65  (scatter block bases;
            # pitch 65 gives the sentinel hi=64 a harmless trash slot)
            w8 = persist.tile([128, 32], dt.int16, tag="w8")
            nc.vector.tensor_scalar(
                w8[:], iota64[:, 0:32], 7, None, op0=Alu.bitwise_and)
            nc.vector.tensor_scalar(
                w8[:], w8[:], 65, None, op0=Alu.mult)
            iotaW8HL = persist.tile([128, 64], dt.int16, tag="iw8")
            nc.vector.tensor_copy(
                iotaW8HL[:].rearrange("p (hl w) -> p hl w", hl=2),
                w8[:].unsqueeze(1).to_broadcast([128, 2, 32]))
            onesbf = persist.tile([128, 8], dt.bfloat16, tag="onesbf")
            nc.vector.memset(onesbf[:], 1.0)

            # ---- persistent per-round successor tables: hi plane | lo plane ----
            hl_t = [persist.tile([128, 2 * WSLICE], dt.int16, tag=f"hl{r}",
                                 name=f"hl{r}")
                    for r in range(ROUNDS)]
            fvt = persist.tile([128, ROUNDS], dt.int16)   # first value / round
            lvt = persist.tile([128, ROUNDS], dt.int16)   # last value / round

            # Double-buffered sample tiles
            vts = [persist.tile([128, N_WORDS], dt.int16, tag=f"vt{i}",
                                name=f"vt{i}")
                   for i in range(2)]

            # ================= Stage A: successor tables =================
            for r in range(ROUNDS):
                vt = vts[r % 2]
                nc.scalar.dma_start(vt[:],
                                    samples_d[r * 128:(r + 1) * 128, :])

                # local index = v - 512k (= v XOR 512k since the window is
                # the 512-aligned block); out-of-window lands in [512, 4095]
                # and is then made negative
                # successor list: nexts[j] = v[j+1], sentinel at the end.
                # (the scatter src must be offset-0 contiguous: the GPSIMD
                # ucode mis-reads odd-offset source slices)
                nexts = work.tile([128, N_WORDS], dt.int16, tag="nexts")
                nc.vector.tensor_copy(nexts[:, 0:N_WORDS - 1], vt[:, 1:N_WORDS])
                nc.vector.memset(nexts[:, N_WORDS - 1:N_WORDS], SENTINEL)

                t = work.tile([128, N_WORDS], dt.int16, tag="t")
                nc.vector.tensor_scalar(
                    t[:], vt[:], wofs_sb[:], None,
                    op0=Alu.subtract)
                hib = work.tile([128, N_WORDS], dt.int16, tag="hib")
                nc.vector.tensor_scalar(
                    hib[:], t[:], WSLICE - 1, 4096,
                    op0=Alu.is_gt, op1=Alu.mult)
                nc.vector.tensor_tensor(
                    t[:], t[:], hib[:], op=Alu.subtract)

                nxt = work.tile([128, WSLICE], dt.int16, tag="nxt")
                nc.gpsimd.local_scatter(
                    nxt[:], nexts[:], t[:],
                    channels=128, num_elems=WSLICE, num_idxs=N_WORDS)

                nc.vector.tensor_scalar(
                    hl_t[r][:, 0:WSLICE], nxt[:], 6, None,
                    op0=Alu.logical_shift_right)
                nc.vector.tensor_scalar(
                    hl_t[r][:, WSLICE:2 * WSLICE], nxt[:], 63, None,
                    op0=Alu.bitwise_and)

                nc.vector.tensor_copy(fvt[:, r:r + 1], vt[:, 0:1])
                nc.vector.tensor_copy(lvt[:, r:r + 1],
                                      vt[:, N_WORDS - 1:N_WORDS])

            # ============ Stage B: histogram matmuls + drain ============
            bigacc = persist.tile([64, 1], dt.float32)
            nc.vector.memset(bigacc[:], 0.0)
            for wg in range(N_WG):
                cps = psum.tile([64, WG * 64], dt.float32, tag="cps")
                for p in range(ROUNDS // 2):
                    if p in GP_POOL_P:
                        # GPSIMD path: one-hots written by local_scatter in
                        # (j, hl, w, l) w-major layout; bf16(1.0) data at
                        # address (w%8)*64 + hi|lo within 8-w chunks.
                        ohS = ohp.tile([128, 2 * 2 * WG * 65], dt.bfloat16,
                                       tag="ohS")
                        ohS4 = ohS[:].rearrange(
                            "q (j hl w l) -> q j hl (w l)", j=2, hl=2,
                            w=WG, l=65)
                        for j in range(2):
                            sc = 2 * p + j
                            idxoh = drp.tile([128, 64], dt.int16,
                                             tag="idxoh")
                            hl_s = hl_t[sc][:].rearrange(
                                "p (hl w) -> p hl w", hl=2)[
                                :, :, wg * WG:(wg + 1) * WG]
                            nc.vector.tensor_tensor(
                                idxoh[:].rearrange("p (hl w) -> p hl w",
                                                   hl=2),
                                hl_s,
                                iotaW8HL[:].rearrange(
                                    "p (hl w) -> p hl w", hl=2),
                                op=Alu.add)
                            for hl in range(2):
                                for c in range(4):
                                    nc.gpsimd.local_scatter(
                                        ohS4[:, j, hl,
                                             c * 520:(c + 1) * 520],
                                        onesbf[:],
                                        idxoh[:, hl * 32 + c * 8:
                                              hl * 32 + c * 8 + 8],
                                        channels=128, num_elems=520,
                                        num_idxs=8)
                        oh8S = ohS[:].bitcast(dt.float8e4).rearrange(
                            "q (j hl w l b) -> q j hl w l b", j=2, hl=2,
                            w=WG, l=65, b=2)
                        for w in range(WG):
                            nc.tensor.matmul(
                                cps[:, w * 64:(w + 1) * 64],
                                oh8S[:, :, 0, w, 0:64, 1],
                                oh8S[:, :, 1, w, 0:64, 1],
                                start=(p == 0 and w % 8 == 0),
                                stop=(p == ROUNDS // 2 - 1 and w % 8 == 7),
                                skip_group_check=True,
                                perf_mode=mybir.MatmulPerfMode.DoubleRow)
                        continue
                    # DVE path: bf16 one-hots, (round j, l, hi|lo, w)
                    # layout; every DVE operand has a packed 2-byte last
                    # dim -> 2x perf mode.
                    oh2 = ohp.tile([128, 2 * 64 * 2 * WG], dt.bfloat16,
                                   tag="oh2")
                    oh4 = oh2[:].rearrange("q (j l hl w) -> q j l hl w",
                                           j=2, l=64, hl=2)
                    ilhw = iotaLHW[:].rearrange("p (l hl w) -> p l hl w",
                                                l=64, hl=2)
                    for j in range(2):
                        sc = 2 * p + j
                        hl_s = hl_t[sc][:].rearrange(
                            "p (hl w) -> p hl w", hl=2)[
                            :, :, wg * WG:(wg + 1) * WG]
                        nc.vector.tensor_tensor(
                            oh4[:, j],
                            hl_s.unsqueeze(1).to_broadcast([128, 64, 2, WG]),
                            ilhw, op=Alu.is_equal)
                    # fp8 byte view: per slot the bf16 bytes are
                    # (0x80, 0x3F)*hit; byte 1 carries 1.875*hit.  The two
                    # j-slices (rounds) are the DoubleRow k-tile pair.
                    oh8 = oh2[:].bitcast(dt.float8e4).rearrange(
                        "q (j l hl w b) -> q j l hl w b", j=2, l=64, hl=2,
                        b=2)
                    for w in range(WG):
                        # start=True zeroes the whole PSUM bank (8 w per
                        # 2KB bank), so only the first matmul into each
                        # bank may set it; everything else accumulates.
                        nc.tensor.matmul(
                            cps[:, w * 64:(w + 1) * 64],
                            oh8[:, :, :, 0, w, 1],
                            oh8[:, :, :, 1, w, 1],
                            start=(p == 0 and w % 8 == 0),
                            stop=(p == ROUNDS // 2 - 1 and w % 8 == 7),
                            skip_group_check=True,
                            perf_mode=mybir.MatmulPerfMode.DoubleRow)

                for h in range(2):
                    hw = WG // 2
                    b3 = b3p.tile([64, hw * 64], dt.float32, tag="b3")
                    bsrc = bslice_d[wg * WG + h * hw:wg * WG + (h + 1) * hw,
                                    :].rearrange("w (h l) -> h w l", h=64)
                    nc.scalar.dma_start(
                        b3[:].rearrange("h (w l) -> h w l", w=hw), bsrc)
                    prod = drp.tile([64, hw * 64], dt.float32, tag="prod")
                    nc.vector.tensor_tensor(
                        prod[:], cps[:, h * hw * 64:(h + 1) * hw * 64], b3[:],
                        op=Alu.mult)
                    red = drp.tile([64, 1], dt.float32, tag="red")
                    nc.vector.tensor_reduce(
                        red[:], prod[:], axis=mybir.AxisListType.X, op=Alu.add)
                    nc.vector.tensor_tensor(
                        bigacc[:], bigacc[:], red[:], op=Alu.add)

            # ============ Stage C: start/end/gold terms ============
            # windowed local index of first/last sample values
            stloc = persist.tile([128, ROUNDS], dt.int16)
            enloc = persist.tile([128, ROUNDS], dt.int16)
            nc.vector.tensor_scalar(
                stloc[:], fvt[:], wofs_sb[:], None, op0=Alu.subtract)
            nc.vector.tensor_scalar(
                enloc[:], lvt[:], wofs_sb[:], None, op0=Alu.subtract)

            def _mini_hist(loc_tile, tag):
                hi0 = persist.tile([128, ROUNDS], dt.int16, tag=f"hi0{tag}")
                lo0 = persist.tile([128, ROUNDS], dt.int16, tag=f"lo0{tag}")
                nc.vector.tensor_scalar(
                    hi0[:], loc_tile[:], 6, None, op0=Alu.logical_shift_right)
                nc.vector.tensor_scalar(
                    lo0[:], loc_tile[:], 63, None, op0=Alu.bitwise_and)
                ohh = persist.tile([128, ROUNDS * 64], dt.float8e4,
                                   tag=f"ohh{tag}")
                ohl = persist.tile([128, ROUNDS * 64], dt.float8e4,
                                   tag=f"ohl{tag}")
                nc.vector.tensor_tensor(
                    ohh[:].rearrange("p (r l) -> p r l", r=ROUNDS),
                    hi0[:].unsqueeze(2).to_broadcast([128, ROUNDS, 64]),
                    iota64[:].unsqueeze(1).to_broadcast([128, ROUNDS, 64]),
                    op=Alu.is_equal)
                nc.vector.tensor_tensor(
                    ohl[:].rearrange("p (r l) -> p r l", r=ROUNDS),
                    lo0[:].unsqueeze(2).to_broadcast([128, ROUNDS, 64]),
                    iota64[:].unsqueeze(1).to_broadcast([128, ROUNDS, 64]),
                    op=Alu.is_equal)
                cmini = psc.tile([64, 64], dt.float32, tag=f"cm{tag}")
                for r in range(ROUNDS):
                    nc.tensor.matmul(
                        cmini[:],
                        ohh[:, r * 64:(r + 1) * 64],
                        ohl[:, r * 64:(r + 1) * 64],
                        start=(r == 0), stop=(r == ROUNDS - 1))
                return cmini

            cst = _mini_hist(stloc, "s")
            cen = _mini_hist(enloc, "e")

            stsb = persist.tile([8, 64], dt.float32, tag="stsb")
            nc.gpsimd.dma_start(
                stsb[:], startv_d[:].rearrange("x (h l) -> (x h) l", h=8))
            ensb = persist.tile([8, 64], dt.float32, tag="ensb")
            nc.gpsimd.dma_start(
                ensb[:], endv_d[:].rearrange("x (h l) -> (x h) l", h=8))

            pst = persist.tile([8, 64], dt.float32, tag="pst")
            nc.vector.tensor_tensor(pst[:], cst[0:8, :], stsb[:], op=Alu.mult)
            stred = persist.tile([8, 1], dt.float32, tag="stred")
            nc.vector.tensor_reduce(stred[:], pst[:], axis=mybir.AxisListType.X, op=Alu.add)

            pen = persist.tile([8, 64], dt.float32, tag="pen")
            nc.vector.tensor_tensor(pen[:], cen[0:8, :], ensb[:], op=Alu.mult)
            enred = persist.tile([8, 1], dt.float32, tag="enred")
            nc.vector.tensor_reduce(enred[:], pen[:], axis=mybir.AxisListType.X, op=Alu.add)

            diagsb = persist.tile([128, 32], dt.float32, tag="diagsb")
            nc.gpsimd.dma_start(
                diagsb[:], diag_d[:].rearrange("x (p c) -> (x p) c", p=128))
            dgred = persist.tile([128, 1], dt.float32, tag="dgred")
            nc.vector.tensor_reduce(dgred[:], diagsb[:], axis=mybir.AxisListType.X, op=Alu.add)

            # ---- partition reductions via PE (dot with ones) ----
            outp = psc.tile([1, 4], dt.float32, tag="outp")
            nc.tensor.matmul(outp[:, 0:1], bigacc[:], ones128[0:64, :],
                             start=True, stop=True)
            nc.tensor.matmul(outp[:, 1:2], stred[:], ones128[0:8, :],
                             start=True, stop=True)
            nc.tensor.matmul(outp[:, 2:3], enred[:], ones128[0:8, :],
                             start=True, stop=True)
            nc.tensor.matmul(outp[:, 3:4], dgred[:], ones128[:],
                             start=True, stop=True)

            outsb = persist.tile([1, 4], dt.float32, tag="outsb")
            nc.vector.tensor_copy(outsb[:], outp[:])
            nc.gpsimd.dma_start(partial_d[:], outsb[:])

    nc.finalize()
    return nc


def _make_in_maps(bigram, start, end, samples):
    samples = samples.astype(np.int16)

    # gold payload: start[0] + end[-1] + superdiagonal of B, summed on device
    diag0 = np.zeros((1, N_WORDS), dtype=np.float32)
    diag0[0, :N_WORDS - 1] = bigram.reshape(-1)[1::N_WORDS + 1][:N_WORDS - 1]
    diag0[0, N_WORDS - 1] = start[0] + end[-1]
    zdiag = np.zeros((1, N_WORDS), dtype=np.float32)

    in_maps = []
    for k in range(N_CORES):
        w0 = k * WSLICE
        in_maps.append({
            "samples": samples,
            "bslice": bigram[w0:w0 + WSLICE, :],
            "wofs": np.full((128, 1), w0, dtype=np.float32),
            "startv": start[w0:w0 + WSLICE].reshape(1, WSLICE),
            "endv": end[w0:w0 + WSLICE].reshape(1, WSLICE),
            "diag": diag0 if k == 0 else zdiag,
        })
    return in_maps


def kernel(bigram, start, end, samples):
    from concourse.bass_utils import run_bass_kernel_spmd

    if "nc" not in _CACHE:
        _CACHE["nc"] = _build_module()
    nc = _CACHE["nc"]

    bigram = np.ascontiguousarray(bigram, dtype=np.float32)
    start = np.ascontiguousarray(start, dtype=np.float32)
    end = np.ascontiguousarray(end, dtype=np.float32)
    samples = np.ascontiguousarray(samples, dtype=np.int32)

    in_maps = _make_in_maps(bigram, start, end, samples)

    res = run_bass_kernel_spmd(nc, in_maps, list(range(N_CORES)))
    parts = np.stack([r["partial"].reshape(4) for r in res.results])

    s_total = float(parts[:, 0].sum() / OH_SCALE
                    + parts[:, 1].sum() + parts[:, 2].sum())
    gold = float(parts[:, 3].sum())
    loss = -gold + s_total / N_SAMPLES
    return (np.float32(loss), np.float32(gold))
